# revision 13
# baseline (speedup 1.0000x reference)
"""Trainium2 Bass kernel for nn_CT_37821482009425 (snntorch Leaky LIF scan).

Reference semantics (bitwise-matched):
    T = clip(t, 1, 5); x = roll(inp, roll_amount, axis=2)
    per step: reset = (mem > T); mem = 0.95*mem + x_t - reset*T; spk = (mem > T)
Output: spikes (1024, 1, 224, 224) float32 in {0, 1}.

Distribution: pure data parallelism — batch 1024 -> 8 cores x 128 (the SBUF
partition dim). Host prep per core: apply the roll and transpose to
time-major so each timestep's H=224 vector is contiguous per partition.

Per-core compute (vector engine, per time step, all stock DVE ops whose
rounding matches the reference exactly):
    v      = scalar_tensor_tensor(mem[t-1], 0.95, x_t, mult, add)
    mem_t  = tensor_tensor(v, r[t-1], subtract)
    r_t    = tensor_scalar(mem_t, T, T, is_gt, mult)        # in {0, T}
r_t doubles as the reset feedback and the spike record (spk = r != 0 on host).
Input DMA (sync engine) and output DMA (scalar engine) run in 8-step slices,
double-buffered against compute in 32-step chunk buffers.
"""

import numpy as np
import concourse.bass as bass
import concourse.mybir as mybir
from concourse.bass_utils import run_bass_kernel_spmd

BETA = 0.95
B, CH = 1024, 224
N_CORES = 8
PB = B // N_CORES  # 128 batches per core = partition dim
H = CH  # per-step vector length (contiguous, time-major)
W = CH  # time steps
WC = 32  # chunk size (SBUF buffer granularity)
SUB = 2  # DMA slice granularity (steps)
N_CHUNK = W // WC
SUBS_PER_CHUNK = WC // SUB
N_SUB = W // SUB

_Alu = mybir.AluOpType

_cache = {}


def _build(T: float):
    nc = bass.Bass(trn_type="TRN2")
    x_d = nc.dram_tensor("x", [PB, W * H], mybir.dt.float32, kind="ExternalInput")
    r_d = nc.dram_tensor("r", [PB, W * H], mybir.dt.float32, kind="ExternalOutput")

    with (
        nc.sbuf_tensor("xt0", [PB, WC * H], mybir.dt.float32) as xt0,
        nc.sbuf_tensor("xt1", [PB, WC * H], mybir.dt.float32) as xt1,
        nc.sbuf_tensor("rt0", [PB, WC * H], mybir.dt.float32) as rt0,
        nc.sbuf_tensor("rt1", [PB, WC * H], mybir.dt.float32) as rt1,
        nc.sbuf_tensor("mcol", [PB, 2 * H], mybir.dt.float32) as mcol,
        nc.sbuf_tensor("vcol", [PB, H], mybir.dt.float32) as vcol,
        nc.semaphore() as in_sem,
        nc.semaphore() as v_sem,
        nc.semaphore() as out_sem,
        nc.Block() as block,
    ):
        xb = [xt0, xt1]
        rb = [rt0, rt1]

        # v_sem: vector increments once per completed SUB-slice (28 total).

        @block.sync
        def _(sync):
            # input DMA, one 8-step slice at a time
            for sb in range(N_SUB):
                c, sl = divmod(sb, SUBS_PER_CHUNK)
                if c >= 2:
                    # xt[c%2] slice sl is free once chunk c-2's compute has
                    # fully consumed that slice (vector bumps v_sem per slice)
                    sync.wait_ge(v_sem, (c - 2) * SUBS_PER_CHUNK + sl + 1)
                sync.dma_start(
                    xb[c % 2][:, sl * SUB * H : (sl + 1) * SUB * H],
                    x_d[:, sb * SUB * H : (sb + 1) * SUB * H],
                ).then_inc(in_sem, 16)

        @block.scalar
        def _(scalar):
            for sb in range(N_SUB):
                c, sl = divmod(sb, SUBS_PER_CHUNK)
                scalar.wait_ge(v_sem, sb + 1)
                scalar.dma_start(
                    r_d[:, sb * SUB * H : (sb + 1) * SUB * H],
                    rb[c % 2][:, sl * SUB * H : (sl + 1) * SUB * H],
                ).then_inc(out_sem, 16)

        @block.vector
        def _(vector):
            for sb in range(N_SUB):
                c, sl = divmod(sb, SUBS_PER_CHUNK)
                vector.wait_ge(in_sem, 16 * (sb + 1))
                if sl == 0 and c >= 2:
                    # rt[c%2] free once all its out-DMA slices (chunk c-2)
                    # completed
                    vector.wait_ge(out_sem, 16 * (c - 1) * SUBS_PER_CHUNK)
                xt, rt = xb[c % 2], rb[c % 2]
                for tl in range(sl * SUB, (sl + 1) * SUB):
                    t = c * WC + tl
                    xcol = xt[:, tl * H : (tl + 1) * H]
                    rcol = rt[:, tl * H : (tl + 1) * H]
                    mc = mcol[:, (t % 2) * H : (t % 2 + 1) * H]
                    if t == 0:
                        nc.vector.tensor_copy(mc, xcol)
                    else:
                        mp = mcol[:, ((t - 1) % 2) * H : ((t - 1) % 2 + 1) * H]
                        if tl == 0:
                            rprev = rb[(c - 1) % 2][:, (WC - 1) * H :]
                        else:
                            rprev = rt[:, (tl - 1) * H : tl * H]
                        nc.vector.scalar_tensor_tensor(
                            vcol[:], mp, BETA, xcol, _Alu.mult, _Alu.add
                        )
                        nc.vector.tensor_tensor(mc, vcol[:], rprev, _Alu.subtract)
                    ts = nc.vector.tensor_scalar(
                        rcol, mc, T, T, _Alu.is_gt, _Alu.mult
                    )
                    if tl % SUB == SUB - 1:
                        ts.then_inc(v_sem, 1)

    return nc


def kernel(inp: np.ndarray, t: np.ndarray, roll_amount) -> np.ndarray:
    T = float(
        np.clip(np.float32(np.asarray(t).reshape(-1)[0]), np.float32(1.0),
                np.float32(5.0))
    )
    roll = int(np.asarray(roll_amount)) % W

    key = (T,)
    if key not in _cache:
        _cache[key] = _build(T)
    nc = _cache[key]

    inp = np.asarray(inp, dtype=np.float32).reshape(B, CH, CH)
    in_maps = []
    for c in range(N_CORES):
        shard = inp[c * PB : (c + 1) * PB]  # (128, H, W)
        shard = np.roll(shard, roll, axis=2)
        # time-major: (128, W, H) contiguous
        x_tm = np.ascontiguousarray(shard.transpose(0, 2, 1)).reshape(PB, W * H)
        in_maps.append({"x": x_tm})

    res = run_bass_kernel_spmd(nc, in_maps, core_ids=list(range(N_CORES)))

    out = np.empty((B, 1, CH, CH), dtype=np.float32)
    for c in range(N_CORES):
        r = res.results[c]["r"].reshape(PB, W, H)  # (b, w, h)
        out[c * PB : (c + 1) * PB, 0] = (r != 0).transpose(0, 2, 1)
    return out
